# revision 13
# baseline (speedup 1.0000x reference)
"""Trainium2 Bass kernel for causal GQA attention (B=2, S=2048, D=2048,
H=32, KVH=8, hd=64) with RoPE and output projection, running SPMD on 8
NeuronCores.

Sharding: tensor-parallel over heads (4-way) x data-parallel over batch
(2-way).  Core c (b = c//4, k = c%4) handles batch b and heads
8k..8k+8 (kv heads 2k, 2k+1).  No collectives: each core computes a
PARTIAL wo product (contraction over its local 512 attention features,
all 2048 output dims) and the host sums the 4 partials per batch.

Layouts: everything lives in transposed [feature, seq] form so that the
head dim (the contraction dim of QK^T) sits on SBUF partitions and no
on-device transposes are required (except a cheap PE transpose for V).
All matmul operands are bf16 (stationary bf16 enables fast weight load
so LDWEIGHTS hides under the matmuls); accumulation is fp32 in PSUM.

Head pairing: q_fin[i] holds head 8k+i (kv head 2k) on partitions 0:64
and head 8k+4+i (kv head 2k+1) on partitions 64:128, so the two packed
score matmuls of a pair use the two DIFFERENT kv heads and the K
projection needs no duplication.

Schedule: the projection work for s-tile t+1 and the wo matmuls for
q-tile t-1 are WOVEN between the attention score groups of q-tile t, so
the PE has dense matmul work while the scalar engine (exp, the
second-busiest engine) chews through the softmax, and exp work starts
as early as possible.  PSUM: score tag 2x2 banks + PV 2x1 + a shared
1-bank tag for projection passes / wo chunks x2 = 8 banks.
"""

import numpy as np

DIM = 2048
S = 2048
B = 2
H = 32
KVH = 8
HD = 64
P = 128
QT = 512        # q tile (free dim of score matmuls)
ROPE_BASE = 10000.0
N_CORES = 8

_CACHE = {}


def _build(s_len=S):
    import concourse.bacc as bacc
    import concourse.tile as tile
    import concourse.mybir as mybir
    from concourse.masks import make_identity

    F32 = mybir.dt.float32
    BF16 = mybir.dt.bfloat16
    Exp = mybir.ActivationFunctionType.Exp

    nqt = s_len // QT      # q tiles
    nkv = s_len // P       # kv tiles of 128
    DK = DIM // P          # 16 contraction tiles for projections
    NXC = 4                # x chunks per s-tile
    OCH = DK // NXC        # 4 contraction 128-tiles per x chunk

    nc = bacc.Bacc("TRN2", target_bir_lowering=False, debug=False,
                   num_devices=N_CORES)

    xT = nc.dram_tensor("xT", [DIM, s_len], BF16, kind="ExternalInput").ap()
    wqT = nc.dram_tensor("wqT", [DIM, 512], BF16, kind="ExternalInput").ap()
    wkT = nc.dram_tensor("wkT", [DIM, 128], BF16, kind="ExternalInput").ap()
    wvT = nc.dram_tensor("wvT", [DIM, 128], BF16, kind="ExternalInput").ap()
    woT = nc.dram_tensor("woT", [512, DIM], BF16, kind="ExternalInput").ap()
    cosT = nc.dram_tensor("cosT", [P, s_len], BF16, kind="ExternalInput").ap()
    sinT = nc.dram_tensor("sinT", [P, s_len], BF16, kind="ExternalInput").ap()
    maskT = nc.dram_tensor("maskT", [P, 4, QT], BF16, kind="ExternalInput").ap()
    out_t = nc.dram_tensor("out_t", [DIM, s_len], BF16,
                           kind="ExternalOutput").ap()

    xT3 = xT.rearrange("(o p) s -> p o s", p=P)
    wqT3 = wqT.rearrange("(o p) f -> p o f", p=P)
    wkT3 = wkT.rearrange("(o p) f -> p o f", p=P)
    wvT3 = wvT.rearrange("(o p) f -> p o f", p=P)
    woT3 = woT.rearrange("(o p) d -> p o d", p=P)
    out3 = out_t.rearrange("(o p) s -> p o s", p=P)

    with tile.TileContext(nc) as tc:
        with (
            tc.tile_pool(name="pers", bufs=1) as pers,
            tc.tile_pool(name="ps", bufs=1, space="PSUM") as ps,
            tc.tile_pool(name="pc", bufs=1) as pc,
        ):
            # ---- persistent tiles ----
            q_fin = [pers.tile([P, s_len], BF16, name=f"q_fin{m}")
                     for m in range(4)]
            k_fin = pers.tile([P, s_len], BF16, name="k_fin")
            v1 = [pers.tile([P, nkv, P], BF16, name=f"v1_{g}")
                  for g in range(2)]
            a_fin = [pers.tile([P, s_len], BF16, name=f"a_fin{i}")
                     for i in range(4)]
            msk = pers.tile([P, 4, QT], BF16, name="msk")
            vT_raw = pers.tile([P, s_len], BF16, name="vT_raw")
            wq_sb = [pers.tile([P, DK, P], BF16, name=f"wq_sb{m}")
                     for m in range(4)]
            wk_sb = pers.tile([P, DK, P], BF16, name="wk_sb")
            wv_sb = pers.tile([P, DK, P], BF16, name="wv_sb")
            wo_sb = pers.tile([P, 4, DIM], BF16, name="wo_sb")
            cos_sb = pers.tile([P, s_len], BF16, name="cos_sb")
            sin_sb = pers.tile([P, s_len], BF16, name="sin_sb")
            ident = pers.tile([P, P], BF16, name="ident")

            # PSUM tags (8 banks): sc2 = 2 tiles x 2 banks (score groups),
            # pv = 2 tiles x 1 bank (PV accum), aq = 2 tiles x 1 bank
            # (projection passes, wo chunks, V transposes).
            def sc2(name):
                return ps.tile([P, 2, QT], F32, tag="sc2", bufs=2, name=name)

            def pvb(name):
                return ps.tile([P, QT], F32, tag="pv", bufs=2, name=name)

            def aqb(name, shape=None, dtype=None):
                return ps.tile(shape or [P, QT], dtype or F32, tag="aq",
                               bufs=2, name=name)

            # ---------------- prologue DMAs ----------------
            # x chunks stream on the sync HWDGE queue; everything else on
            # the scalar HWDGE queue so the x stream is never stuck behind
            # 6 MB of weights.
            xsl = {}

            def x_load(st):
                for cn in range(NXC):
                    t_ = pc.tile([P, OCH, QT], BF16, tag="xsl",
                                 bufs=2 * NXC, name=f"x{st}_{cn}")
                    nc.sync.dma_start(
                        t_[:], xT3[:, cn * OCH:(cn + 1) * OCH,
                                   st * QT:(st + 1) * QT])
                    xsl[st, cn] = t_

            x_load(0)
            nc.scalar.dma_start(wk_sb[:], wkT3[:])
            nc.scalar.dma_start(wv_sb[:], wvT3[:])
            nc.scalar.dma_start(cos_sb[:], cosT[:])
            nc.scalar.dma_start(sin_sb[:], sinT[:])
            for m in range(4):
                nc.scalar.dma_start(wq_sb[m][:],
                                    wqT3[:, :, m * P:(m + 1) * P])
            nc.scalar.dma_start(msk[:], maskT[:])
            nc.scalar.dma_start(wo_sb[:], woT3[:])
            ident_f = pc.tile([P, P], F32, name="ident_f")
            make_identity(nc, ident_f[:])
            nc.vector.tensor_copy(ident[:], ident_f[:])
            ones3 = pc.tile([P, nkv, HD], F32, name="ones3")
            nc.vector.memset(ones3[:], 1.0)
            for g in range(2):
                nc.vector.tensor_copy(v1[g][:, :, 0:HD], ones3[:])

            # ---------------- stage-A unit generators ----------------
            def rope_chain(dst, src_ps, ssl, on_act):
                raw = pc.tile([P, QT], BF16, tag="raw", bufs=4, name="raw")
                if on_act:
                    nc.scalar.copy(raw[:], src_ps)
                else:
                    nc.vector.tensor_copy(raw[:], src_ps)
                # rotate-half: partition swap via 4 small SBUF->SBUF DMAs
                # on the (otherwise idle) gpsimd SWDGE queue, keeping the
                # sync HWDGE queue free for the x/out streams.  The sign
                # of the rotation is folded into sinT host-side.
                rot = pc.tile([P, QT], BF16, tag="rot", bufs=3, name="rot")
                for hh in range(2):
                    base = hh * HD
                    nc.gpsimd.dma_start(rot[base:base + 32, :],
                                        raw[base + 32:base + 64, :])
                    nc.gpsimd.dma_start(rot[base + 32:base + 64, :],
                                        raw[base:base + 32, :])
                nc.vector.tensor_mul(rot[:], rot[:], sin_sb[:, ssl])
                nc.vector.tensor_mul(raw[:], raw[:], cos_sb[:, ssl])
                nc.vector.tensor_add(dst[:, ssl], raw[:], rot[:])

            # pass order within a s-tile: k first, then v, then q pairs —
            # the first attention group of the next phase consumes k_fin,
            # v1 and q_fin[0], in that order, so their ropes/transposes
            # must complete earliest (else the first score matmul
            # head-of-line-blocks the PE queue at the phase boundary).
            PASS_ORDER = (4, 5, 0, 1, 2, 3)

            def a_pass(st, which):
                """One projection pass for s-tile st: 16 accumulating
                matmuls into a single psum bank, then drain.  which:
                0-3 = q pair, 4 = k, 5 = v."""
                ssl = slice(st * QT, (st + 1) * QT)
                acc = aqb(f"ap_{st}_{which}")
                w = (wq_sb[which] if which < 4 else
                     (wk_sb if which == 4 else wv_sb))
                for o in range(DK):
                    nc.tensor.matmul(acc[:], w[:, o, :],
                                     xsl[st, o // OCH][:, o % OCH, :],
                                     start=(o == 0), stop=(o == DK - 1))
                if which < 4:
                    rope_chain(q_fin[which], acc[:], ssl, on_act=False)
                elif which == 4:
                    rope_chain(k_fin, acc[:], ssl, on_act=False)
                else:
                    nc.scalar.copy(vT_raw[:, ssl], acc[:])
                    for j in range(4 * st, 4 * st + 4):
                        pst = aqb(f"pst{j}", [P, P], BF16)
                        nc.tensor.transpose(
                            pst[:], vT_raw[:, j * P:(j + 1) * P], ident[:])
                        for g in range(2):
                            nc.vector.tensor_copy(
                                v1[g][:, j, HD:P],
                                pst[:, g * HD:(g + 1) * HD])

            # ---------------- attention unit generators ----------------
            prs = [slice(0, HD), slice(HD, P)]
            pair_state = {}

            def attn_group(t, m, g2):
                """Score pair + exp + (mask) + lagged PV for kv group g2
                of head pair m at q tile t."""
                st8 = pair_state[t, m]
                qsl = slice(t * QT, (t + 1) * QT)
                pss = [sc2(f"ss_{t}_{m}_{g2}_{hf}") for hf in range(2)]
                for i in range(2):
                    j = 2 * g2 + i
                    for hf in range(2):
                        nc.tensor.matmul(
                            pss[hf][:, i, :],
                            k_fin[prs[hf], j * P:(j + 1) * P],
                            q_fin[m][prs[hf], qsl],
                            start=True, stop=True)
                e2 = []
                for hf in range(2):
                    e = pc.tile([P, 2, QT], BF16, tag="exp", bufs=8,
                                name="e2")
                    nc.scalar.activation(e[:], pss[hf][:], Exp, scale=0.125)
                    cpair = g2 - 2 * t
                    if cpair >= 0:
                        nc.vector.tensor_mul(
                            e[:], e[:], msk[:, 2 * cpair:2 * cpair + 2, :])
                    e2.append(e)
                st8["e"].append(e2)
                if g2 >= 1:
                    _pv_flush(t, m, g2 - 1)

            def _pv_flush(t, m, gp):
                st8 = pair_state[t, m]
                last_j = 4 * t + 3
                for i in range(2):
                    j = 2 * gp + i
                    for hf in range(2):
                        nc.tensor.matmul(
                            st8["pv"][hf][:], v1[hf][:, j, :],
                            st8["e"][gp][hf][:, i, :],
                            start=(j == 0), stop=(j == last_j))

            def attn_norm(t, m):
                st8 = pair_state[t, m]
                _pv_flush(t, m, 2 * (t + 1) - 1)
                qsl = slice(t * QT, (t + 1) * QT)
                for hf in range(2):
                    recip = pc.tile([HD, QT], F32, tag="recip", bufs=2,
                                    name="recip")
                    nc.vector.reciprocal_approx_fast(
                        recip[:], st8["pv"][hf][0:HD, :])
                    nc.vector.tensor_mul(
                        a_fin[m][hf * HD:(hf + 1) * HD, qsl],
                        st8["pv"][hf][HD:P, :], recip[:])

            def wo_pair(t, dp, tail=False):
                """Partial wo for q tile t, output d-pair dp (2 x 128
                dims): contract over the local 512 attn features.  Two
                1-bank psum chunks drain into one SBUF tile -> one DMA."""
                qsl = slice(t * QT, (t + 1) * QT)
                ot = pc.tile([P, 2, QT], BF16, tag="ot", bufs=3, name="ot")
                for dd in range(2):
                    d = 2 * dp + dd
                    pw = aqb(f"wo_{t}_{d}")
                    for f in range(4):
                        nc.tensor.matmul(
                            pw[:], wo_sb[:, f, d * P:(d + 1) * P],
                            a_fin[f][:, qsl], start=(f == 0), stop=(f == 3))
                    if tail and dd == 1:
                        nc.scalar.copy(ot[:, dd, :], pw[:])
                    else:
                        nc.vector.tensor_copy(ot[:, dd, :], pw[:])
                nc.sync.dma_start(out3[:, 2 * dp:2 * dp + 2, qsl], ot[:])

            # ---------------- woven schedule ----------------
            # prologue: s-tile 0 projections
            for w in PASS_ORDER:
                a_pass(0, w)

            for t in range(nqt):
                # c-units: attention groups + per-pair normalize
                c_units = []
                for m in range(4):
                    pair_state[t, m] = {
                        "pv": [pvb(f"pv_{t}_{m}_{hf}") for hf in range(2)],
                        "e": []}
                    for g2 in range(2 * (t + 1)):
                        c_units.append(
                            lambda t=t, m=m, g2=g2: attn_group(t, m, g2))
                    c_units.append(lambda t=t, m=m: attn_norm(t, m))
                # filler units: wo chunks of tile t-1, projection passes
                # of s-tile t+1 (x chunks DMA-kicked first)
                f_units = []
                if t + 1 < nqt:
                    f_units.append(lambda st=t + 1: x_load(st))
                    for w in PASS_ORDER:
                        f_units.append(lambda st=t + 1, w=w: a_pass(st, w))
                if t >= 1:
                    for dp in range(DK // 2):
                        f_units.append(lambda t=t - 1, dp=dp: wo_pair(t, dp))
                # interleave: spread fillers evenly between c-units
                nf, ncu = len(f_units), len(c_units)
                fi = 0
                for ci, cu in enumerate(c_units):
                    cu()
                    while fi < nf and fi * ncu < (ci + 1) * nf:
                        f_units[fi]()
                        fi += 1
                while fi < nf:
                    f_units[fi]()
                    fi += 1
            # tail: last tile's wo; second drain of each pair on the
            # (now idle) scalar engine so drains overlap the matmuls
            for dp in range(DK // 2):
                wo_pair(nqt - 1, dp, tail=True)

    nc.compile()
    return nc


def _prep_inputs(x, position_ids, wq, wk, wv, wo):
    import ml_dtypes

    bf16 = ml_dtypes.bfloat16
    x = np.asarray(x, dtype=np.float32)
    pos = np.asarray(position_ids).reshape(-1).astype(np.int64)
    wqf = np.asarray(wq, dtype=np.float32)
    wkf = np.asarray(wk, dtype=np.float32)
    wvf = np.asarray(wv, dtype=np.float32)
    wof = np.asarray(wo, dtype=np.float32)

    inv = 1.0 / (ROPE_BASE ** (np.arange(0, HD, 2, dtype=np.float32) / HD))
    freqs = np.outer(pos.astype(np.float32), inv)  # [S, 32]
    pidx = np.arange(P) % 32
    sign = np.where((np.arange(P) % HD) < 32, -1.0, 1.0).astype(np.float32)
    cosT = np.ascontiguousarray(np.cos(freqs)[:, pidx].T).astype(bf16)
    sinT = np.ascontiguousarray(
        np.sin(freqs)[:, pidx].T * sign[:, None]).astype(bf16)

    pg = np.arange(P)[:, None, None]
    cg = np.arange(4)[None, :, None]
    fg = np.arange(QT)[None, None, :]
    maskT = ((fg - pg - 128 * cg) >= 0).astype(bf16)

    xT = [np.ascontiguousarray(x[b].T).astype(bf16) for b in range(B)]

    in_maps = []
    for c in range(N_CORES):
        b, k = c // 4, c % 4
        # q columns: pair i holds head 8k+i (cols 0:64 of the pair) and
        # head 8k+4+i (cols 64:128)
        qcols = np.concatenate(
            [np.arange(64 * (8 * k + i + 4 * hf), 64 * (8 * k + i + 4 * hf) + 64)
             for i in range(4) for hf in range(2)])
        wqT_loc = np.ascontiguousarray(wqf[qcols].T).astype(bf16)
        kvcols = np.arange(64 * 2 * k, 64 * (2 * k + 2))
        wkT_loc = np.ascontiguousarray(wkf[kvcols].T).astype(bf16)
        wvT_loc = np.ascontiguousarray(wvf[kvcols].T).astype(bf16)
        # wo rows in the a_fin feature order (f = 128*i + 64*hf + d)
        woT_loc = np.ascontiguousarray(wof[:, qcols].T).astype(bf16)
        in_maps.append({
            "xT": xT[b],
            "wqT": wqT_loc,
            "wkT": wkT_loc,
            "wvT": wvT_loc,
            "woT": woT_loc,
            "cosT": cosT,
            "sinT": sinT,
            "maskT": maskT,
        })
    return in_maps


LAST_EXEC_NS = None


def kernel(x, position_ids, wq, wk, wv, wo, _trace=False):
    import time

    from concourse import bass_utils

    if "nc" not in _CACHE:
        _CACHE["nc"] = _build()
    nc = _CACHE["nc"]

    in_maps = _prep_inputs(x, position_ids, wq, wk, wv, wo)
    res = None
    for attempt in range(3):
        try:
            res = bass_utils.run_bass_kernel_spmd(
                nc, in_maps, core_ids=list(range(N_CORES)), trace=_trace)
            break
        except Exception:
            # transient device hiccups (e.g. NRT_EXEC_UNIT_UNRECOVERABLE
            # after rapid back-to-back runs) usually clear on retry
            if attempt == 2:
                raise
            time.sleep(20 * (attempt + 1))

    global LAST_EXEC_NS
    LAST_EXEC_NS = res.exec_time_ns

    out = np.zeros((B, S, DIM), dtype=np.float32)
    for c in range(N_CORES):
        b = c // 4
        out[b] += res.results[c]["out_t"].astype(np.float32).T
    return out


# revision 16
# speedup vs baseline: 1.0127x; 1.0127x over previous
"""Trainium2 Bass kernel for causal GQA attention (B=2, S=2048, D=2048,
H=32, KVH=8, hd=64) with RoPE and output projection, running SPMD on 8
NeuronCores.

Sharding: tensor-parallel over heads (4-way) x data-parallel over batch
(2-way).  Core c (b = c//4, k = c%4) handles batch b and heads
8k..8k+8 (kv heads 2k, 2k+1).  No collectives: each core computes a
PARTIAL wo product (contraction over its local 512 attention features,
all 2048 output dims) and the host sums the 4 partials per batch.

Layouts: everything lives in transposed [feature, seq] form so that the
head dim (the contraction dim of QK^T) sits on SBUF partitions and no
on-device transposes are required (except a cheap PE transpose for V).
All matmul operands are bf16 (stationary bf16 enables fast weight load
so LDWEIGHTS hides under the matmuls); accumulation is fp32 in PSUM.

Head pairing: q_fin[i] holds head 8k+i (kv head 2k) on partitions 0:64
and head 8k+4+i (kv head 2k+1) on partitions 64:128, so the two packed
score matmuls of a pair use the two DIFFERENT kv heads and the K
projection needs no duplication.

Schedule: the projection work for s-tile t+1 and the wo matmuls for
q-tile t-1 are WOVEN between the attention score groups of q-tile t, so
the PE has dense matmul work while the scalar engine (exp, the
second-busiest engine) chews through the softmax, and exp work starts
as early as possible.  PSUM: score tag 2x2 banks + PV 2x1 + a shared
1-bank tag for projection passes / wo chunks x2 = 8 banks.
"""

import numpy as np

DIM = 2048
S = 2048
B = 2
H = 32
KVH = 8
HD = 64
P = 128
QT = 512        # q tile (free dim of score matmuls)
ROPE_BASE = 10000.0
N_CORES = 8

_CACHE = {}


def _build(s_len=S):
    import concourse.bacc as bacc
    import concourse.tile as tile
    import concourse.mybir as mybir
    from concourse.masks import make_identity

    F32 = mybir.dt.float32
    BF16 = mybir.dt.bfloat16
    Exp = mybir.ActivationFunctionType.Exp

    nqt = s_len // QT      # q tiles
    nkv = s_len // P       # kv tiles of 128
    DK = DIM // P          # 16 contraction tiles for projections
    NXC = 4                # x chunks per s-tile
    OCH = DK // NXC        # 4 contraction 128-tiles per x chunk

    nc = bacc.Bacc("TRN2", target_bir_lowering=False, debug=False,
                   num_devices=N_CORES)

    xT = nc.dram_tensor("xT", [DIM, s_len], BF16, kind="ExternalInput").ap()
    wqT = nc.dram_tensor("wqT", [DIM, 512], BF16, kind="ExternalInput").ap()
    wkT = nc.dram_tensor("wkT", [DIM, 128], BF16, kind="ExternalInput").ap()
    wvT = nc.dram_tensor("wvT", [DIM, 128], BF16, kind="ExternalInput").ap()
    woT = nc.dram_tensor("woT", [512, DIM], BF16, kind="ExternalInput").ap()
    cosT = nc.dram_tensor("cosT", [P, s_len], BF16, kind="ExternalInput").ap()
    sinT = nc.dram_tensor("sinT", [P, s_len], BF16, kind="ExternalInput").ap()
    maskT = nc.dram_tensor("maskT", [P, 4, QT], BF16, kind="ExternalInput").ap()
    out_t = nc.dram_tensor("out_t", [DIM, s_len], BF16,
                           kind="ExternalOutput").ap()

    xT3 = xT.rearrange("(o p) s -> p o s", p=P)
    wqT3 = wqT.rearrange("(o p) f -> p o f", p=P)
    wkT3 = wkT.rearrange("(o p) f -> p o f", p=P)
    wvT3 = wvT.rearrange("(o p) f -> p o f", p=P)
    woT3 = woT.rearrange("(o p) d -> p o d", p=P)
    out3 = out_t.rearrange("(o p) s -> p o s", p=P)

    with tile.TileContext(nc) as tc:
        with (
            tc.tile_pool(name="pers", bufs=1) as pers,
            tc.tile_pool(name="ps", bufs=1, space="PSUM") as ps,
            tc.tile_pool(name="pc", bufs=1) as pc,
        ):
            # ---- persistent tiles ----
            q_fin = [pers.tile([P, s_len], BF16, name=f"q_fin{m}")
                     for m in range(4)]
            k_fin = pers.tile([P, s_len], BF16, name="k_fin")
            v1 = [pers.tile([P, nkv, P], BF16, name=f"v1_{g}")
                  for g in range(2)]
            a_fin = [pers.tile([P, s_len], BF16, name=f"a_fin{i}")
                     for i in range(4)]
            msk = pers.tile([P, 4, QT], BF16, name="msk")
            vT_raw = pers.tile([P, s_len], BF16, name="vT_raw")
            wq_sb = [pers.tile([P, DK, P], BF16, name=f"wq_sb{m}")
                     for m in range(4)]
            wk_sb = pers.tile([P, DK, P], BF16, name="wk_sb")
            wv_sb = pers.tile([P, DK, P], BF16, name="wv_sb")
            wo_sb = pers.tile([P, 4, DIM], BF16, name="wo_sb")
            cos_sb = pers.tile([P, s_len], BF16, name="cos_sb")
            sin_sb = pers.tile([P, s_len], BF16, name="sin_sb")
            ident = pers.tile([P, P], BF16, name="ident")

            # PSUM tags (8 banks): sc2 = 2 tiles x 2 banks (score groups),
            # pv = 2 tiles x 1 bank (PV accum), aq = 2 tiles x 1 bank
            # (projection passes, wo chunks, V transposes).
            def sc2(name):
                return ps.tile([P, 2, QT], F32, tag="sc2", bufs=2, name=name)

            def pvb(name):
                return ps.tile([P, QT], F32, tag="pv", bufs=2, name=name)

            def aqb(name, shape=None, dtype=None):
                return ps.tile(shape or [P, QT], dtype or F32, tag="aq",
                               bufs=2, name=name)

            # ---------------- prologue DMAs ----------------
            # x chunks stream on the sync HWDGE queue; everything else on
            # the scalar HWDGE queue so the x stream is never stuck behind
            # 6 MB of weights.
            xsl = {}

            def x_load(st):
                for cn in range(NXC):
                    t_ = pc.tile([P, OCH, QT], BF16, tag="xsl",
                                 bufs=2 * NXC, name=f"x{st}_{cn}")
                    nc.sync.dma_start(
                        t_[:], xT3[:, cn * OCH:(cn + 1) * OCH,
                                   st * QT:(st + 1) * QT])
                    xsl[st, cn] = t_

            x_load(0)
            nc.scalar.dma_start(wk_sb[:], wkT3[:])
            nc.scalar.dma_start(wv_sb[:], wvT3[:])
            nc.scalar.dma_start(cos_sb[:], cosT[:])
            nc.scalar.dma_start(sin_sb[:], sinT[:])
            for m in range(4):
                nc.scalar.dma_start(wq_sb[m][:],
                                    wqT3[:, :, m * P:(m + 1) * P])
            nc.scalar.dma_start(msk[:], maskT[:])
            nc.scalar.dma_start(wo_sb[:], woT3[:])
            ident_f = pc.tile([P, P], F32, name="ident_f")
            make_identity(nc, ident_f[:])
            nc.vector.tensor_copy(ident[:], ident_f[:])
            ones3 = pc.tile([P, nkv, HD], F32, name="ones3")
            nc.vector.memset(ones3[:], 1.0)
            for g in range(2):
                nc.vector.tensor_copy(v1[g][:, :, 0:HD], ones3[:])

            # ---------------- stage-A unit generators ----------------
            def rope_chain(dst, src_ps, ssl, on_act):
                raw = pc.tile([P, QT], BF16, tag="raw", bufs=4, name="raw")
                if on_act:
                    nc.scalar.copy(raw[:], src_ps)
                else:
                    nc.vector.tensor_copy(raw[:], src_ps)
                # rotate-half: partition swap via 4 small SBUF->SBUF DMAs
                # on the (otherwise idle) gpsimd SWDGE queue, keeping the
                # sync HWDGE queue free for the x/out streams.  The sign
                # of the rotation is folded into sinT host-side.
                rot = pc.tile([P, QT], BF16, tag="rot", bufs=3, name="rot")
                for hh in range(2):
                    base = hh * HD
                    nc.gpsimd.dma_start(rot[base:base + 32, :],
                                        raw[base + 32:base + 64, :])
                    nc.gpsimd.dma_start(rot[base + 32:base + 64, :],
                                        raw[base:base + 32, :])
                nc.vector.tensor_mul(rot[:], rot[:], sin_sb[:, ssl])
                nc.vector.tensor_mul(raw[:], raw[:], cos_sb[:, ssl])
                nc.vector.tensor_add(dst[:, ssl], raw[:], rot[:])

            # pass order within a s-tile: k first, then v, then q pairs —
            # the first attention group of the next phase consumes k_fin,
            # v1 and q_fin[0], in that order, so their ropes/transposes
            # must complete earliest (else the first score matmul
            # head-of-line-blocks the PE queue at the phase boundary).
            PASS_ORDER = (0, 1, 2, 3, 4, 5)

            def a_pass(st, which):
                """One projection pass for s-tile st: 16 accumulating
                matmuls into a single psum bank, then drain.  which:
                0-3 = q pair, 4 = k, 5 = v."""
                ssl = slice(st * QT, (st + 1) * QT)
                acc = aqb(f"ap_{st}_{which}")
                w = (wq_sb[which] if which < 4 else
                     (wk_sb if which == 4 else wv_sb))
                for o in range(DK):
                    nc.tensor.matmul(acc[:], w[:, o, :],
                                     xsl[st, o // OCH][:, o % OCH, :],
                                     start=(o == 0), stop=(o == DK - 1))
                if which < 4:
                    rope_chain(q_fin[which], acc[:], ssl, on_act=False)
                elif which == 4:
                    rope_chain(k_fin, acc[:], ssl, on_act=False)
                else:
                    nc.scalar.copy(vT_raw[:, ssl], acc[:])
                    for j in range(4 * st, 4 * st + 4):
                        pst = aqb(f"pst{j}", [P, P], BF16)
                        nc.tensor.transpose(
                            pst[:], vT_raw[:, j * P:(j + 1) * P], ident[:])
                        for g in range(2):
                            nc.vector.tensor_copy(
                                v1[g][:, j, HD:P],
                                pst[:, g * HD:(g + 1) * HD])

            # ---------------- attention unit generators ----------------
            prs = [slice(0, HD), slice(HD, P)]
            pair_state = {}

            def attn_group(t, m, g2):
                """Score pair + exp + (mask) + lagged PV for kv group g2
                of head pair m at q tile t."""
                st8 = pair_state[t, m]
                qsl = slice(t * QT, (t + 1) * QT)
                pss = [sc2(f"ss_{t}_{m}_{g2}_{hf}") for hf in range(2)]
                for i in range(2):
                    j = 2 * g2 + i
                    for hf in range(2):
                        nc.tensor.matmul(
                            pss[hf][:, i, :],
                            k_fin[prs[hf], j * P:(j + 1) * P],
                            q_fin[m][prs[hf], qsl],
                            start=True, stop=True)
                e2 = []
                for hf in range(2):
                    e = pc.tile([P, 2, QT], BF16, tag="exp", bufs=8,
                                name="e2")
                    nc.scalar.activation(e[:], pss[hf][:], Exp, scale=0.125)
                    cpair = g2 - 2 * t
                    if cpair >= 0:
                        nc.vector.tensor_mul(
                            e[:], e[:], msk[:, 2 * cpair:2 * cpair + 2, :])
                    e2.append(e)
                st8["e"].append(e2)
                if g2 >= 1:
                    _pv_flush(t, m, g2 - 1)

            def _pv_flush(t, m, gp):
                st8 = pair_state[t, m]
                last_j = 4 * t + 3
                for i in range(2):
                    j = 2 * gp + i
                    for hf in range(2):
                        nc.tensor.matmul(
                            st8["pv"][hf][:], v1[hf][:, j, :],
                            st8["e"][gp][hf][:, i, :],
                            start=(j == 0), stop=(j == last_j))

            def attn_norm(t, m):
                st8 = pair_state[t, m]
                _pv_flush(t, m, 2 * (t + 1) - 1)
                qsl = slice(t * QT, (t + 1) * QT)
                for hf in range(2):
                    recip = pc.tile([HD, QT], F32, tag="recip", bufs=2,
                                    name="recip")
                    nc.vector.reciprocal_approx_fast(
                        recip[:], st8["pv"][hf][0:HD, :])
                    nc.vector.tensor_mul(
                        a_fin[m][hf * HD:(hf + 1) * HD, qsl],
                        st8["pv"][hf][HD:P, :], recip[:])

            def wo_pair(t, dp, tail=False):
                """Partial wo for q tile t, output d-pair dp (2 x 128
                dims): contract over the local 512 attn features.  Two
                1-bank psum chunks drain into one SBUF tile -> one DMA."""
                qsl = slice(t * QT, (t + 1) * QT)
                ot = pc.tile([P, 2, QT], BF16, tag="ot", bufs=3, name="ot")
                for dd in range(2):
                    d = 2 * dp + dd
                    pw = aqb(f"wo_{t}_{d}")
                    for f in range(4):
                        nc.tensor.matmul(
                            pw[:], wo_sb[:, f, d * P:(d + 1) * P],
                            a_fin[f][:, qsl], start=(f == 0), stop=(f == 3))
                    if tail and dd == 1:
                        nc.scalar.copy(ot[:, dd, :], pw[:])
                    else:
                        nc.vector.tensor_copy(ot[:, dd, :], pw[:])
                nc.sync.dma_start(out3[:, 2 * dp:2 * dp + 2, qsl], ot[:])

            # ---------------- woven schedule ----------------
            # prologue: s-tile 0 projections
            for w in PASS_ORDER:
                a_pass(0, w)

            for t in range(nqt):
                # c-units: attention groups + per-pair normalize
                c_units = []
                for m in range(4):
                    pair_state[t, m] = {
                        "pv": [pvb(f"pv_{t}_{m}_{hf}") for hf in range(2)],
                        "e": []}
                    for g2 in range(2 * (t + 1)):
                        c_units.append(
                            lambda t=t, m=m, g2=g2: attn_group(t, m, g2))
                    c_units.append(lambda t=t, m=m: attn_norm(t, m))
                # filler units: wo chunks of tile t-1, projection passes
                # of s-tile t+1 (x chunks DMA-kicked first)
                f_units = []
                if t + 1 < nqt:
                    f_units.append(lambda st=t + 1: x_load(st))
                    for w in PASS_ORDER:
                        f_units.append(lambda st=t + 1, w=w: a_pass(st, w))
                if t >= 1:
                    for dp in range(DK // 2):
                        f_units.append(lambda t=t - 1, dp=dp: wo_pair(t, dp))
                # interleave: spread fillers evenly between c-units
                nf, ncu = len(f_units), len(c_units)
                fi = 0
                for ci, cu in enumerate(c_units):
                    cu()
                    while fi < nf and fi * ncu < (ci + 1) * nf:
                        f_units[fi]()
                        fi += 1
                while fi < nf:
                    f_units[fi]()
                    fi += 1
            # tail: last tile's wo; second drain of each pair on the
            # (now idle) scalar engine so drains overlap the matmuls
            for dp in range(DK // 2):
                wo_pair(nqt - 1, dp, tail=True)

    nc.compile()
    return nc


def _prep_inputs(x, position_ids, wq, wk, wv, wo):
    import ml_dtypes

    bf16 = ml_dtypes.bfloat16
    x = np.asarray(x, dtype=np.float32)
    pos = np.asarray(position_ids).reshape(-1).astype(np.int64)
    wqf = np.asarray(wq, dtype=np.float32)
    wkf = np.asarray(wk, dtype=np.float32)
    wvf = np.asarray(wv, dtype=np.float32)
    wof = np.asarray(wo, dtype=np.float32)

    inv = 1.0 / (ROPE_BASE ** (np.arange(0, HD, 2, dtype=np.float32) / HD))
    freqs = np.outer(pos.astype(np.float32), inv)  # [S, 32]
    pidx = np.arange(P) % 32
    sign = np.where((np.arange(P) % HD) < 32, -1.0, 1.0).astype(np.float32)
    cosT = np.ascontiguousarray(np.cos(freqs)[:, pidx].T).astype(bf16)
    sinT = np.ascontiguousarray(
        np.sin(freqs)[:, pidx].T * sign[:, None]).astype(bf16)

    pg = np.arange(P)[:, None, None]
    cg = np.arange(4)[None, :, None]
    fg = np.arange(QT)[None, None, :]
    maskT = ((fg - pg - 128 * cg) >= 0).astype(bf16)

    xT = [np.ascontiguousarray(x[b].T).astype(bf16) for b in range(B)]

    in_maps = []
    for c in range(N_CORES):
        b, k = c // 4, c % 4
        # q columns: pair i holds head 8k+i (cols 0:64 of the pair) and
        # head 8k+4+i (cols 64:128)
        qcols = np.concatenate(
            [np.arange(64 * (8 * k + i + 4 * hf), 64 * (8 * k + i + 4 * hf) + 64)
             for i in range(4) for hf in range(2)])
        wqT_loc = np.ascontiguousarray(wqf[qcols].T).astype(bf16)
        kvcols = np.arange(64 * 2 * k, 64 * (2 * k + 2))
        wkT_loc = np.ascontiguousarray(wkf[kvcols].T).astype(bf16)
        wvT_loc = np.ascontiguousarray(wvf[kvcols].T).astype(bf16)
        # wo rows in the a_fin feature order (f = 128*i + 64*hf + d)
        woT_loc = np.ascontiguousarray(wof[:, qcols].T).astype(bf16)
        in_maps.append({
            "xT": xT[b],
            "wqT": wqT_loc,
            "wkT": wkT_loc,
            "wvT": wvT_loc,
            "woT": woT_loc,
            "cosT": cosT,
            "sinT": sinT,
            "maskT": maskT,
        })
    return in_maps


LAST_EXEC_NS = None


def kernel(x, position_ids, wq, wk, wv, wo, _trace=False):
    import time

    from concourse import bass_utils

    if "nc" not in _CACHE:
        _CACHE["nc"] = _build()
    nc = _CACHE["nc"]

    in_maps = _prep_inputs(x, position_ids, wq, wk, wv, wo)
    res = None
    for attempt in range(3):
        try:
            res = bass_utils.run_bass_kernel_spmd(
                nc, in_maps, core_ids=list(range(N_CORES)), trace=_trace)
            break
        except Exception:
            # transient device hiccups (e.g. NRT_EXEC_UNIT_UNRECOVERABLE
            # after rapid back-to-back runs) usually clear on retry
            if attempt == 2:
                raise
            time.sleep(20 * (attempt + 1))

    global LAST_EXEC_NS
    LAST_EXEC_NS = res.exec_time_ns

    out = np.zeros((B, S, DIM), dtype=np.float32)
    for c in range(N_CORES):
        b = c // 4
        out[b] += res.results[c]["out_t"].astype(np.float32).T
    return out
